# revision 23
# baseline (speedup 1.0000x reference)
"""Distributed MoE (top-1 routing) for 8 Trainium2 NeuronCores.

Strategy (expert parallel, as in the sharding hint):
  - Router (x @ Wr + br, argmax) is computed on the host in f64 as part of
    sharding: token->expert assignment decides which core gets each token.
    (min top1-top2 logit gap for this distribution is ~1e-5, far above f32
    rounding noise, so host argmax == jax f32 argmax.)
  - Core e holds expert e's weights (W1[e], b1[e], W2[e], b2[e]) resident in
    SBUF (bf16 weights) and runs a dense 2-layer FFN over the tokens routed
    to expert e, feature-major (transposed) so both matmuls need no on-chip
    transposes:
        hT = relu(W1^T x^T + b1)   via matmul(lhsT=W1 chunk, rhs=xT chunk)
        yT = W2^T hT + b2          via matmul(lhsT=W2 chunk, rhs=hT chunk)
  - Tokens are padded per-core to a common capacity C (max expert count
    rounded up to 128) so one NEFF serves all 8 cores (SPMD).
  - Host scatters per-core outputs back to the original token order.

Matmuls run in bf16 (1 cycle/row on the PE vs 4 for fp32) with fp32 PSUM
accumulation; biases and outputs stay fp32.
"""

import math
from functools import lru_cache

import ml_dtypes
import numpy as np

N_TOKENS = 16384
D_IN = 1024
D_HID = 4096
D_OUT = 1024
N_EXPERTS = 8
N_CORES = 8
P = 128
TB = 512  # token block (PSUM free-dim limit for f32 accumulation)

_BF16 = ml_dtypes.bfloat16

# Set by run when MOE_TRACE=1; test.py reads exec_time_ns from here.
LAST_RESULTS = None


def _block_sizes(C: int):
    """Split C token columns into near-equal blocks of <= TB.

    Equal sizes keep every matmul's moving dim large enough (>=410 for the
    relevant C range) that LDWEIGHTS stays hidden behind the matmul stream;
    a short remainder block would run LDWEIGHTS-bound instead."""
    nb = math.ceil(C / TB)
    base, rem = divmod(C, nb)
    return [base + (1 if i < rem else 0) for i in range(nb)]


# Number of warm-up matmuls on zeroed SBUF issued while weights stream in:
# keeps the PE busy through the HAM activity window so real matmuls start at
# 2.4 GHz instead of the cold 1.2 GHz half-rate.
N_WARM = 24


@lru_cache(maxsize=2)
def _build_nc(C: int):
    import concourse.mybir as mybir
    from bass_rust import add_dep_helper
    from concourse import bacc
    from concourse.tile import TileContext

    F32 = mybir.dt.float32
    BF16 = mybir.dt.bfloat16
    RELU = mybir.ActivationFunctionType.Relu
    IDENT = mybir.ActivationFunctionType.Identity

    sizes = _block_sizes(C)
    nb = len(sizes)
    KC1 = D_IN // P     # 8   contraction chunks, layer 1
    MC1 = D_HID // P    # 32  output chunks, layer 1
    KC2 = D_HID // P    # 32  contraction chunks, layer 2
    MC2 = D_OUT // P    # 8   output chunks, layer 2

    nc = bacc.Bacc("TRN2", target_bir_lowering=False, debug=False)

    xt = nc.dram_tensor("xt", [nb, P, KC1, TB], BF16, kind="ExternalInput")
    w1 = nc.dram_tensor("w1", [D_IN, D_HID], BF16, kind="ExternalInput")
    w2 = nc.dram_tensor("w2", [D_HID, D_OUT], BF16, kind="ExternalInput")
    b1t = nc.dram_tensor("b1t", [P, MC1], F32, kind="ExternalInput")
    b2t = nc.dram_tensor("b2t", [P, MC2], F32, kind="ExternalInput")
    yt = nc.dram_tensor("yt", [nb, MC2, P, TB], F32, kind="ExternalOutput")

    with TileContext(nc) as tc:
        with (
            tc.tile_pool(name="weights", bufs=1) as wpool,
            tc.tile_pool(name="xin", bufs=3) as xpool,
            tc.tile_pool(name="hid", bufs=1) as hpool,
            tc.tile_pool(name="yout", bufs=4) as ypool,
            tc.tile_pool(name="ps_h", bufs=3, space="PSUM") as psh,
            tc.tile_pool(name="ps_y", bufs=3, space="PSUM") as psy,
        ):
            warm_sb = wpool.tile([P, 128 + TB], BF16, tag="warm")
            nc.vector.memset(warm_sb[:], 0.0)
            warm_ps = psh.tile([P, TB], F32, tag="ph")
            for i in range(N_WARM):
                nc.tensor.matmul(
                    warm_ps[:], warm_sb[:, :P], warm_sb[:, P:],
                    start=(i == 0), stop=(i == N_WARM - 1),
                )
            warm_out = wpool.tile([P, TB], F32, tag="warmout")
            nc.scalar.copy(warm_out[:], warm_ps[:])

            b1_sb = wpool.tile([P, MC1], F32, tag="b1")
            nc.sync.dma_start(b1_sb[:], b1t[:, :])
            b2_sb = wpool.tile([P, MC2], F32, tag="b2")
            nc.sync.dma_start(b2_sb[:], b2t[:, :])

            # W1 is on the critical path (layer-1 matmuls sweep all of it
            # within the first couple of microseconds): load it via the
            # scalar engine's HW-DGE queues, which sit idle this early.
            # W2 isn't needed until layer 2 of block 0 (~60us in): gate it
            # on W1's completion so it doesn't steal HBM bandwidth.
            # Quarter-split every W1 row-chunk and issue quarter-major:
            # quarter q holds hid columns [q*1024, (q+1)*1024), i.e. exactly
            # what layer-1 output chunks hc = 8q..8q+7 read. Loading q=0 of
            # all 8 row-chunks first (2 MB) lets the first hc groups start
            # ~20us before the full 8 MB of W1 has landed; later quarters
            # stream in behind the compute.
            NQ = 4
            QW = D_HID // NQ
            w1_sb = [wpool.tile([P, D_HID], BF16, tag=f"w1_{kc}", name=f"w1_{kc}")
                     for kc in range(KC1)]
            w1_last = None
            for q in range(NQ):
                for kc in range(KC1):
                    w1_last = nc.sync.dma_start(
                        w1_sb[kc][:, q * QW:(q + 1) * QW],
                        w1[kc * P:(kc + 1) * P, q * QW:(q + 1) * QW],
                    )
            w2_sb = []
            for kc in range(KC2):
                t = wpool.tile([P, D_OUT], BF16, tag=f"w2_{kc}")
                d = nc.gpsimd.dma_start(t[:], w2[kc * P:(kc + 1) * P, :])
                add_dep_helper(d.ins, w1_last.ins, sync=True,
                               reason="W2 after W1 (HBM priority)")
                w2_sb.append(t)

            for b in range(nb):
                tb = sizes[b]
                xt_sb = xpool.tile([P, KC1, TB], BF16, tag="xt")
                d = nc.sync.dma_start(xt_sb[:], xt[b, :, :, :])
                if b > 0:
                    # keep token prefetches off the HBM bus while W1 streams
                    add_dep_helper(d.ins, w1_last.ins, sync=True,
                                   reason="xt prefetch after W1 (HBM priority)")

                ht_sb = hpool.tile([P, KC2, TB], BF16, tag="ht")
                for hc in range(MC1):
                    ps = psh.tile([P, TB], F32, tag="ph")
                    for kc in range(KC1):
                        nc.tensor.matmul(
                            ps[:, :tb],
                            w1_sb[kc][:, hc * P:(hc + 1) * P],
                            xt_sb[:, kc, :tb],
                            start=(kc == 0),
                            stop=(kc == KC1 - 1),
                        )
                    nc.scalar.activation(
                        ht_sb[:, hc, :tb], ps[:, :tb], RELU,
                        bias=b1_sb[:, hc:hc + 1],
                    )

                for oc in range(MC2):
                    ps = psy.tile([P, TB], F32, tag="py")
                    for kc in range(KC2):
                        nc.tensor.matmul(
                            ps[:, :tb],
                            w2_sb[kc][:, oc * P:(oc + 1) * P],
                            ht_sb[:, kc, :tb],
                            start=(kc == 0),
                            stop=(kc == KC2 - 1),
                        )
                    y_sb = ypool.tile([P, TB], F32, tag="y")
                    nc.scalar.activation(
                        y_sb[:, :tb], ps[:, :tb], IDENT,
                        bias=b2_sb[:, oc:oc + 1],
                    )
                    nc.sync.dma_start(yt[b, oc, :, :tb], y_sb[:, :tb])

    nc.finalize()
    return nc


def kernel(x, Wr, br, W1, b1, W2, b2):
    import os

    from concourse.bass_utils import run_bass_kernel_spmd

    global LAST_RESULTS

    x = np.asarray(x, dtype=np.float32)
    Wr = np.asarray(Wr, dtype=np.float32)
    br = np.asarray(br, dtype=np.float32)
    W1 = np.asarray(W1, dtype=np.float32)
    b1 = np.asarray(b1, dtype=np.float32)
    W2 = np.asarray(W2, dtype=np.float32)
    b2 = np.asarray(b2, dtype=np.float32)

    # --- Router on host (part of sharding): f64 matches f32 argmax safely.
    logits = x.astype(np.float64) @ Wr.astype(np.float64) + br.astype(np.float64)
    eidx = np.argmax(logits, axis=1)
    counts = np.bincount(eidx, minlength=N_EXPERTS)

    C = max(int(counts.max()), P)
    sizes = _block_sizes(C)
    nb = len(sizes)
    offs = np.concatenate([[0], np.cumsum(sizes)])

    order = np.argsort(eidx, kind="stable")
    starts = np.zeros(N_EXPERTS + 1, dtype=np.int64)
    np.cumsum(counts, out=starts[1:])

    x_bf = x.astype(_BF16)

    in_maps = []
    tok_ids = []
    for e in range(N_EXPERTS):
        idx = order[starts[e]:starts[e + 1]]
        tok_ids.append(idx)
        xp = np.zeros((C, D_IN), dtype=_BF16)
        xp[:len(idx)] = x_bf[idx]
        # per block: [tb, D_IN] -> [P(ki), KC1(ko), tb] with d_in = ko*P + ki
        xt = np.zeros((nb, P, D_IN // P, TB), dtype=_BF16)
        for b in range(nb):
            seg = xp[offs[b]:offs[b + 1]]
            xt[b, :, :, :sizes[b]] = seg.reshape(-1, D_IN // P, P).transpose(2, 1, 0)
        in_maps.append({
            "xt": xt,
            "w1": W1[e].astype(_BF16),
            "w2": W2[e].astype(_BF16),
            "b1t": np.ascontiguousarray(b1[e].reshape(D_HID // P, P).T),
            "b2t": np.ascontiguousarray(b2[e].reshape(D_OUT // P, P).T),
        })

    nc = _build_nc(C)

    trace = os.environ.get("MOE_TRACE", "0") == "1"
    kwargs = {}
    if trace:
        kwargs = {"trace": True, "trace_cores": list(range(N_CORES))}
    res = run_bass_kernel_spmd(nc, in_maps, core_ids=list(range(N_CORES)), **kwargs)
    LAST_RESULTS = res

    out = np.zeros((N_TOKENS, D_OUT), dtype=np.float32)
    for e in range(N_EXPERTS):
        ytb = res.results[e]["yt"]  # [nb, MC2, P, TB] f32
        y = np.empty((C, D_OUT), dtype=np.float32)
        for b in range(nb):
            # [MC2, P, tb] -> [tb, MC2*P]
            y[offs[b]:offs[b + 1]] = (
                ytb[b, :, :, :sizes[b]].transpose(2, 0, 1).reshape(sizes[b], D_OUT)
            )
        idx = tok_ids[e]
        out[idx] = y[:len(idx)]
    return out


# revision 25
# speedup vs baseline: 1.0312x; 1.0312x over previous
"""Distributed MoE (top-1 routing) for 8 Trainium2 NeuronCores.

Strategy (expert parallel, as in the sharding hint):
  - Router (x @ Wr + br, argmax) is computed on the host in f64 as part of
    sharding: token->expert assignment decides which core gets each token.
    (min top1-top2 logit gap for this distribution is ~1e-5, far above f32
    rounding noise, so host argmax == jax f32 argmax.)
  - Core e holds expert e's weights (W1[e], b1[e], W2[e], b2[e]) resident in
    SBUF (bf16 weights) and runs a dense 2-layer FFN over the tokens routed
    to expert e, feature-major (transposed) so both matmuls need no on-chip
    transposes:
        hT = relu(W1^T x^T + b1)   via matmul(lhsT=W1 chunk, rhs=xT chunk)
        yT = W2^T hT + b2          via matmul(lhsT=W2 chunk, rhs=hT chunk)
  - Tokens are padded per-core to a common capacity C (max expert count
    rounded up to 128) so one NEFF serves all 8 cores (SPMD).
  - Host scatters per-core outputs back to the original token order.

Matmuls run in bf16 (1 cycle/row on the PE vs 4 for fp32) with fp32 PSUM
accumulation; biases and outputs stay fp32.
"""

import math
from functools import lru_cache

import ml_dtypes
import numpy as np

N_TOKENS = 16384
D_IN = 1024
D_HID = 4096
D_OUT = 1024
N_EXPERTS = 8
N_CORES = 8
P = 128
TB = 512  # token block (PSUM free-dim limit for f32 accumulation)

_BF16 = ml_dtypes.bfloat16

# Set by run when MOE_TRACE=1; test.py reads exec_time_ns from here.
LAST_RESULTS = None


def _block_sizes(C: int):
    """Split C token columns into near-equal blocks of <= TB.

    Equal sizes keep every matmul's moving dim large enough (>=410 for the
    relevant C range) that LDWEIGHTS stays hidden behind the matmul stream;
    a short remainder block would run LDWEIGHTS-bound instead."""
    nb = math.ceil(C / TB)
    base, rem = divmod(C, nb)
    return [base + (1 if i < rem else 0) for i in range(nb)]


# Number of warm-up matmuls on zeroed SBUF issued while weights stream in:
# keeps the PE busy through the HAM activity window so real matmuls start at
# 2.4 GHz instead of the cold 1.2 GHz half-rate.
N_WARM = 24


@lru_cache(maxsize=2)
def _build_nc(C: int):
    import concourse.mybir as mybir
    from bass_rust import add_dep_helper
    from concourse import bacc
    from concourse.tile import TileContext

    F32 = mybir.dt.float32
    BF16 = mybir.dt.bfloat16
    RELU = mybir.ActivationFunctionType.Relu
    IDENT = mybir.ActivationFunctionType.Identity

    sizes = _block_sizes(C)
    nb = len(sizes)
    KC1 = D_IN // P     # 8   contraction chunks, layer 1
    MC1 = D_HID // P    # 32  output chunks, layer 1
    KC2 = D_HID // P    # 32  contraction chunks, layer 2
    MC2 = D_OUT // P    # 8   output chunks, layer 2

    nc = bacc.Bacc("TRN2", target_bir_lowering=False, debug=False)

    xt = nc.dram_tensor("xt", [nb, P, KC1, TB], BF16, kind="ExternalInput")
    w1 = nc.dram_tensor("w1", [D_IN, D_HID], BF16, kind="ExternalInput")
    w2 = nc.dram_tensor("w2", [D_HID, D_OUT], BF16, kind="ExternalInput")
    b1t = nc.dram_tensor("b1t", [P, MC1], F32, kind="ExternalInput")
    b2t = nc.dram_tensor("b2t", [P, MC2], F32, kind="ExternalInput")
    yt = nc.dram_tensor("yt", [nb, MC2, P, TB], F32, kind="ExternalOutput")

    with TileContext(nc) as tc:
        with (
            tc.tile_pool(name="weights", bufs=1) as wpool,
            tc.tile_pool(name="xin", bufs=3) as xpool,
            tc.tile_pool(name="hid", bufs=1) as hpool,
            tc.tile_pool(name="yout", bufs=4) as ypool,
            tc.tile_pool(name="ps_h", bufs=3, space="PSUM") as psh,
            tc.tile_pool(name="ps_y", bufs=3, space="PSUM") as psy,
        ):
            warm_sb = wpool.tile([P, 128 + TB], BF16, tag="warm")
            nc.vector.memset(warm_sb[:], 0.0)
            warm_ps = psh.tile([P, TB], F32, tag="ph")
            for i in range(N_WARM):
                nc.tensor.matmul(
                    warm_ps[:], warm_sb[:, :P], warm_sb[:, P:],
                    start=(i == 0), stop=(i == N_WARM - 1),
                )
            warm_out = wpool.tile([P, TB], F32, tag="warmout")
            nc.scalar.copy(warm_out[:], warm_ps[:])

            # Block 0's tokens go first on the sync queue — the first layer-1
            # matmul needs them along with W1 quarter 0.
            xt0_sb = xpool.tile([P, KC1, TB], BF16, tag="xt", name="xt0")
            nc.sync.dma_start(xt0_sb[:], xt[0, :, :, :])

            b1_sb = wpool.tile([P, MC1], F32, tag="b1")
            nc.sync.dma_start(b1_sb[:], b1t[:, :])
            b2_sb = wpool.tile([P, MC2], F32, tag="b2")
            nc.sync.dma_start(b2_sb[:], b2t[:, :])

            # W1 is on the critical path (layer-1 matmuls sweep all of it
            # within the first couple of microseconds): load it via the
            # scalar engine's HW-DGE queues, which sit idle this early.
            # W2 isn't needed until layer 2 of block 0 (~60us in): gate it
            # on W1's completion so it doesn't steal HBM bandwidth.
            # Quarter-split every W1 row-chunk and issue quarter-major:
            # quarter q holds hid columns [q*1024, (q+1)*1024), i.e. exactly
            # what layer-1 output chunks hc = 8q..8q+7 read. Loading q=0 of
            # all 8 row-chunks first (2 MB) lets the first hc groups start
            # ~20us before the full 8 MB of W1 has landed; later quarters
            # stream in behind the compute.
            NQ = 4
            QW = D_HID // NQ
            w1_sb = [wpool.tile([P, D_HID], BF16, tag=f"w1_{kc}", name=f"w1_{kc}")
                     for kc in range(KC1)]
            w1_last = None
            for q in range(NQ):
                for kc in range(KC1):
                    w1_last = nc.sync.dma_start(
                        w1_sb[kc][:, q * QW:(q + 1) * QW],
                        w1[kc * P:(kc + 1) * P, q * QW:(q + 1) * QW],
                    )
            w2_sb = []
            for kc in range(KC2):
                t = wpool.tile([P, D_OUT], BF16, tag=f"w2_{kc}")
                d = nc.gpsimd.dma_start(t[:], w2[kc * P:(kc + 1) * P, :])
                add_dep_helper(d.ins, w1_last.ins, sync=True,
                               reason="W2 after W1 (HBM priority)")
                w2_sb.append(t)

            for b in range(nb):
                tb = sizes[b]
                if b == 0:
                    xt_sb = xt0_sb
                else:
                    xt_sb = xpool.tile([P, KC1, TB], BF16, tag="xt")
                    d = nc.sync.dma_start(xt_sb[:], xt[b, :, :, :])
                    # keep token prefetches off the HBM bus while W1 streams
                    add_dep_helper(d.ins, w1_last.ins, sync=True,
                                   reason="xt prefetch after W1 (HBM priority)")

                ht_sb = hpool.tile([P, KC2, TB], BF16, tag="ht")
                for hc in range(MC1):
                    ps = psh.tile([P, TB], F32, tag="ph")
                    for kc in range(KC1):
                        nc.tensor.matmul(
                            ps[:, :tb],
                            w1_sb[kc][:, hc * P:(hc + 1) * P],
                            xt_sb[:, kc, :tb],
                            start=(kc == 0),
                            stop=(kc == KC1 - 1),
                        )
                    nc.scalar.activation(
                        ht_sb[:, hc, :tb], ps[:, :tb], RELU,
                        bias=b1_sb[:, hc:hc + 1],
                    )

                for oc in range(MC2):
                    ps = psy.tile([P, TB], F32, tag="py")
                    for kc in range(KC2):
                        nc.tensor.matmul(
                            ps[:, :tb],
                            w2_sb[kc][:, oc * P:(oc + 1) * P],
                            ht_sb[:, kc, :tb],
                            start=(kc == 0),
                            stop=(kc == KC2 - 1),
                        )
                    y_sb = ypool.tile([P, TB], F32, tag="y")
                    nc.scalar.activation(
                        y_sb[:, :tb], ps[:, :tb], IDENT,
                        bias=b2_sb[:, oc:oc + 1],
                    )
                    nc.sync.dma_start(yt[b, oc, :, :tb], y_sb[:, :tb])

    nc.finalize()
    return nc


def kernel(x, Wr, br, W1, b1, W2, b2):
    import os

    from concourse.bass_utils import run_bass_kernel_spmd

    global LAST_RESULTS

    x = np.asarray(x, dtype=np.float32)
    Wr = np.asarray(Wr, dtype=np.float32)
    br = np.asarray(br, dtype=np.float32)
    W1 = np.asarray(W1, dtype=np.float32)
    b1 = np.asarray(b1, dtype=np.float32)
    W2 = np.asarray(W2, dtype=np.float32)
    b2 = np.asarray(b2, dtype=np.float32)

    # --- Router on host (part of sharding): f64 matches f32 argmax safely.
    logits = x.astype(np.float64) @ Wr.astype(np.float64) + br.astype(np.float64)
    eidx = np.argmax(logits, axis=1)
    counts = np.bincount(eidx, minlength=N_EXPERTS)

    C = max(int(counts.max()), P)
    sizes = _block_sizes(C)
    nb = len(sizes)
    offs = np.concatenate([[0], np.cumsum(sizes)])

    order = np.argsort(eidx, kind="stable")
    starts = np.zeros(N_EXPERTS + 1, dtype=np.int64)
    np.cumsum(counts, out=starts[1:])

    x_bf = x.astype(_BF16)

    in_maps = []
    tok_ids = []
    for e in range(N_EXPERTS):
        idx = order[starts[e]:starts[e + 1]]
        tok_ids.append(idx)
        xp = np.zeros((C, D_IN), dtype=_BF16)
        xp[:len(idx)] = x_bf[idx]
        # per block: [tb, D_IN] -> [P(ki), KC1(ko), tb] with d_in = ko*P + ki
        xt = np.zeros((nb, P, D_IN // P, TB), dtype=_BF16)
        for b in range(nb):
            seg = xp[offs[b]:offs[b + 1]]
            xt[b, :, :, :sizes[b]] = seg.reshape(-1, D_IN // P, P).transpose(2, 1, 0)
        in_maps.append({
            "xt": xt,
            "w1": W1[e].astype(_BF16),
            "w2": W2[e].astype(_BF16),
            "b1t": np.ascontiguousarray(b1[e].reshape(D_HID // P, P).T),
            "b2t": np.ascontiguousarray(b2[e].reshape(D_OUT // P, P).T),
        })

    nc = _build_nc(C)

    trace = os.environ.get("MOE_TRACE", "0") == "1"
    kwargs = {}
    if trace:
        kwargs = {"trace": True, "trace_cores": list(range(N_CORES))}
    res = run_bass_kernel_spmd(nc, in_maps, core_ids=list(range(N_CORES)), **kwargs)
    LAST_RESULTS = res

    out = np.zeros((N_TOKENS, D_OUT), dtype=np.float32)
    for e in range(N_EXPERTS):
        ytb = res.results[e]["yt"]  # [nb, MC2, P, TB] f32
        y = np.empty((C, D_OUT), dtype=np.float32)
        for b in range(nb):
            # [MC2, P, tb] -> [tb, MC2*P]
            y[offs[b]:offs[b + 1]] = (
                ytb[b, :, :, :sizes[b]].transpose(2, 0, 1).reshape(sizes[b], D_OUT)
            )
        idx = tok_ids[e]
        out[idx] = y[:len(idx)]
    return out
